# revision 1
# baseline (speedup 1.0000x reference)
"""Trainium2 Bass kernel for nn_COAttention (trilinear co-attention).

Math (per batch, masks are all-ones by problem spec, bias cancels in softmax):
  S    = C@w4C + (Q@w4Q)^T + (C*w4mlu)@Q^T          [Lc, Lq]
  S1   = softmax(S, axis=q) ; S2 = softmax(S, axis=c)
  A    = S1@Q ; Bt = (S1@S2^T)@C = S1@(S2^T@C)      (reassociated)
  out  = concat([C, A, C*A, C*Bt], -1)

Device formulation (single exp pass; exp(sub1) factors cancel in the S2 path):
  E2[c,q] = exp(sub2[c,q] + sub0[c])     (sub0 as per-partition ACT bias)
  w[q]    = exp(sub1[q])                 (host-computed, tiny)
  T'      = (E2^T @ C) / (E2^T @ 1)      == S2^T@C exactly
  [Ab|Bb|r] = E2 @ [Q*w | T'*w | w] ; A = Ab/r ; Bt = Bb/r
Matmul orientations: M1 (d-contraction) uses C^T/Qm^T via DMA-xbar transposed
loads; Gt (c-contraction) uses E2-natural; M3 (q-contraction) uses E2^T built
by PE transposes. C rows ride the xbar 16-row-strip interleaved order
(HW: natural c = i*128 + p); host packs q-side inputs to the matching
natural order, so every DMA is contiguous.

Host prep (0.05% of FLOPs): sub0=C@w4C, w=exp(Q@w4Q), Qm=Q*w4mlu, Qw=Q*w,
bf16 casts. Output: device returns [A|C*A|C*Bt] bf16; host prepends exact C.

Sharding: data-parallel over batch, 2 batches per core on 8 cores.
"""

import os
import sys

if "/opt/trn_rl_repo" not in sys.path:
    sys.path.insert(0, "/opt/trn_rl_repo")

import numpy as np
import ml_dtypes

from concourse import bacc, bass, mybir, tile
from concourse.bass_utils import run_bass_kernel_spmd
from concourse.masks import make_identity

F32 = mybir.dt.float32
BF16 = mybir.dt.bfloat16
EXP = mybir.ActivationFunctionType.Exp
COPY = mybir.ActivationFunctionType.Copy
MULT = mybir.AluOpType.mult
ADD = mybir.AluOpType.add
AX = mybir.AxisListType.X

B, Lc, Lq, D = 16, 2048, 512, 128
NCORES = 8
BPC = B // NCORES          # batches per core
NTC = Lc // 128            # 16 c-tiles
NTQ = Lq // 128            # 4 q-tiles

_NC_CACHE = {}
LAST_RESULT = None

# On HW the DRAM-source xbar transpose is tiling-natural: transposed free
# position f holds row f. (CoreSim models a strip-interleave here instead --
# HW-verified identity wins; see scratch/t_hwmap.py.)
_QIDX = np.empty((128, NTQ), dtype=np.int64)
for _x in range(128):
    for _j in range(NTQ):
        _QIDX[_x, _j] = _j * 128 + _x


def _body(tc, nc, Cd, Qmd, Qwd, Sub0d, Wd, OUT):
    with (
        tc.tile_pool(name="const", bufs=1) as constp,
        tc.tile_pool(name="io", bufs=2) as iop,
        tc.tile_pool(name="big", bufs=2) as bigp,
        tc.tile_pool(name="small", bufs=2) as smallp,
        tc.tile_pool(name="ps_s", bufs=2, space="PSUM") as ps_s,
        tc.tile_pool(name="ps_f", bufs=2, space="PSUM") as ps_f,
        tc.tile_pool(name="ps_t", bufs=2, space="PSUM") as ps_t,
        tc.tile_pool(name="ps_g", bufs=2, space="PSUM") as ps_g,
    ):
        ident = constp.tile([128, 128], BF16)
        make_identity(nc, ident[:])

        st = [dict() for _ in range(BPC)]

        def ph_load(b):
            s = st[b]
            # natural loads, then SBUF-source xbar transposes (the fast path)
            s["qm_sb"] = iop.tile([128, NTQ, 128], BF16, tag="qm_sb", name="qm_sb")
            nc.scalar.dma_start(s["qm_sb"][:],
                                Qmd[b].rearrange("(t p) d -> p t d", p=128))
            s["c_bf"] = iop.tile([128, NTC, 128], BF16, tag="c_bf", name="c_bf")
            nc.scalar.dma_start(s["c_bf"][:],
                                Cd[b].rearrange("(t p) d -> p t d", p=128))
            s["qmt"] = bigp.tile([128, NTQ, 128], BF16, tag="qmt", name="qmt")
            nc.sync.dma_start_transpose(s["qmt"][:], s["qm_sb"][:])
            s["ct"] = bigp.tile([128, NTC, 128], BF16, tag="ct", name="ct")
            nc.sync.dma_start_transpose(s["ct"][:], s["c_bf"][:])
            s["rhs_t"] = bigp.tile([128, NTQ, 257], BF16, tag="rhs", name="rhs")
            nc.scalar.dma_start(s["rhs_t"][:, :, 0:128], Qwd[b][:, :, :])
            s["sub0_col"] = smallp.tile([128, NTC], F32, tag="sub0", name="sub0")
            nc.scalar.dma_start(s["sub0_col"][:], Sub0d[b][:, :])
            s["w_col"] = smallp.tile([128, NTQ], F32, tag="wcol", name="wcol")
            nc.scalar.dma_start(s["w_col"][:], Wd[b][:, :])
            s["e2n"] = bigp.tile([128, NTC, 512], BF16, tag="e2n", name="e2n")
            s["e2t"] = bigp.tile([128, NTQ, NTC, 128], BF16, tag="e2t", name="e2t")
            s["spart"] = smallp.tile([128, NTQ, 4], F32, tag="spart", name="spart")

        def ph_m1_group(b, k):
            # S matmuls + exp for c-tiles 4k..4k+3 -> E2 natural [c-part, q-free]
            s = st[b]
            for m in range(4):
                i = k * 4 + m
                s_ps = ps_s.tile([128, 512], F32, tag="s", name="s")
                nc.tensor.matmul(s_ps[:], lhsT=s["ct"][:, i, :], rhs=s["qmt"][:],
                                 start=True, stop=True)
                nc.scalar.activation(s["e2n"][:, i, :], s_ps[:], EXP,
                                     bias=s["sub0_col"][:, i : i + 1])

        def ph_tgt_group(b, k):
            # E2^T via PE transposes (+ col-sum accum) for c-tiles 4k..4k+3,
            # then the Gt accumulation matmuls for the same tiles.
            s = st[b]
            if k == 0:
                s["gt_ps"] = ps_g.tile([128, 512], F32, tag="g", name="g")
            for j in range(NTQ):
                t_ps = ps_t.tile([128, 4, 128], BF16, tag="t", name="t")
                for m in range(4):
                    i = k * 4 + m
                    nc.tensor.transpose(t_ps[:, m, :],
                                        s["e2n"][:, i, j * 128 : (j + 1) * 128],
                                        ident[:])
                dst = s["e2t"][:, j, k * 4 : (k + 1) * 4, :]
                nc.vector.tensor_scalar(
                    out=dst, in0=t_ps[:], scalar1=1.0, scalar2=None,
                    op0=MULT, op1=ADD,
                    accum_out=s["spart"][:, j, k : k + 1])
            for m in range(4):
                i = k * 4 + m
                nc.tensor.matmul(s["gt_ps"][:], lhsT=s["c_bf"][:, i, :],
                                 rhs=s["e2n"][:, i, :],
                                 start=(i == 0), stop=(i == NTC - 1))

        def ph_trhs(b):
            s = st[b]
            gt_bf = bigp.tile([128, 512], BF16, tag="gtbf", name="gtbf")
            nc.vector.tensor_copy(gt_bf[:], s["gt_ps"][:])
            s_col = smallp.tile([128, NTQ], F32, tag="scol", name="scol")
            nc.vector.reduce_sum(s_col[:], s["spart"][:], axis=AX)
            rs_col = smallp.tile([128, NTQ], F32, tag="rscol", name="rscol")
            nc.vector.reciprocal(rs_col[:], s_col[:])
            ws_col = smallp.tile([128, NTQ], F32, tag="wscol", name="wscol")
            nc.vector.tensor_mul(ws_col[:], s["w_col"][:], rs_col[:])

            gt_tp = ps_t.tile([128, 4, 128], BF16, tag="t", name="t")
            for j in range(NTQ):
                nc.tensor.transpose(gt_tp[:, j, :],
                                    gt_bf[:, j * 128 : (j + 1) * 128], ident[:])
            for j in range(NTQ):
                nc.vector.tensor_scalar_mul(s["rhs_t"][:, j, 128:256],
                                            gt_tp[:, j, :],
                                            ws_col[:, j : j + 1])
                nc.vector.tensor_copy(s["rhs_t"][:, j, 256:257],
                                      s["w_col"][:, j : j + 1])

        def ph_m3(b):
            s = st[b]
            out_sb = bigp.tile([128, NTC, 384], BF16, tag="osb", name="osb")
            for i in range(NTC):
                f_ps = ps_f.tile([128, 257], F32, tag="f", name="f")
                for j in range(NTQ):
                    nc.tensor.matmul(f_ps[:], lhsT=s["e2t"][:, j, i, :],
                                     rhs=s["rhs_t"][:, j, :],
                                     start=(j == 0), stop=(j == NTQ - 1))
                rr = smallp.tile([128, 1], F32, tag="rr", name="rr")
                nc.vector.reciprocal(rr[:], f_ps[:, 256:257])
                ab = smallp.tile([128, 256], BF16, tag="ab", name="ab")
                nc.scalar.activation(ab[:], f_ps[:, 0:256], COPY, scale=rr[:])
                nc.vector.tensor_copy(out_sb[:, i, 0:128], ab[:, 0:128])      # A
                nc.gpsimd.tensor_mul(out_sb[:, i, 128:256], ab[:, 0:128],
                                     s["c_bf"][:, i, :])                      # C*A
                nc.vector.tensor_mul(out_sb[:, i, 256:384], ab[:, 128:256],
                                     s["c_bf"][:, i, :])                      # C*Bt
            nc.sync.dma_start(OUT[b].rearrange("(t p) e -> p t e", p=128),
                              out_sb[:])

        # software-pipelined schedule: the exp stream (ACT) is the long pole
        # up front; PE transpose/Gt groups ride in its shadow, zipped per
        # 4-c-tile group; both epilogues run at the end with a 3-engine split.
        ph_load(0)
        ph_load(1)
        for k in range(4):
            ph_m1_group(0, k)
            ph_tgt_group(0, k)
        ph_trhs(0)
        for k in range(4):
            ph_m1_group(1, k)
            ph_tgt_group(1, k)
        ph_trhs(1)
        ph_m3(0)
        ph_m3(1)


def _build_nc(n_iters=1):
    nc = bacc.Bacc("TRN2", target_bir_lowering=False, debug=False)
    Cd = nc.declare_dram_parameter("C_bf", [BPC, Lc, D], BF16, isOutput=False)
    Qmd = nc.declare_dram_parameter("Qm_bf", [BPC, Lq, D], BF16, isOutput=False)
    Qwd = nc.declare_dram_parameter("QwP_bf", [BPC, 128, NTQ, D], BF16,
                                    isOutput=False)
    Sub0d = nc.declare_dram_parameter("sub0c_f", [BPC, 128, NTC], F32,
                                      isOutput=False)
    Wd = nc.declare_dram_parameter("wcol_f", [BPC, 128, NTQ], F32,
                                   isOutput=False)
    OUT = nc.declare_dram_parameter("OUT", [BPC, Lc, 3 * D], BF16, isOutput=True)
    with tile.TileContext(nc) as tc:
        if n_iters == 1:
            _body(tc, nc, Cd, Qmd, Qwd, Sub0d, Wd, OUT)
        else:
            hints = (mybir.EngineType.PE, mybir.EngineType.DVE,
                     mybir.EngineType.Activation, mybir.EngineType.Pool,
                     mybir.EngineType.SP)
            with tc.For_i(0, n_iters, 1, hint_engines=hints):
                _body(tc, nc, Cd, Qmd, Qwd, Sub0d, Wd, OUT)
    nc.compile()
    return nc


def get_nc():
    if "nc" not in _NC_CACHE:
        _NC_CACHE["nc"] = _build_nc()
    return _NC_CACHE["nc"]


def kernel(C, Q, Cmask=None, Qmask=None, w4C=None, w4Q=None, w4mlu=None,
           bias=None, **_unused):
    """Full inputs in, full output out. Masks are all-ones (problem spec);
    bias is a scalar added to S pre-softmax, which cancels in both softmaxes."""
    global LAST_RESULT
    bf = ml_dtypes.bfloat16
    C = np.asarray(C, dtype=np.float32)
    Q = np.asarray(Q, dtype=np.float32)
    w4C = np.asarray(w4C, dtype=np.float32).reshape(D)
    w4Q = np.asarray(w4Q, dtype=np.float32).reshape(D)
    w4mlu = np.asarray(w4mlu, dtype=np.float32).reshape(D)

    # tiny host prep: rank-1 bias terms + input scalings (0.05% of FLOPs)
    sub0 = C @ w4C                                   # [B, Lc]
    w = np.exp(Q @ w4Q)                              # [B, Lq]
    Qm = (Q * w4mlu).astype(bf)                      # [B, Lq, D]
    Qw = Q * w[:, :, None]                           # [B, Lq, D]
    C_bf = C.astype(bf)
    sub0c = np.ascontiguousarray(sub0.reshape(B, NTC, 128).transpose(0, 2, 1))
    wcol = np.ascontiguousarray(w[:, _QIDX])                      # [B,128,NTQ]
    QwP = np.ascontiguousarray(Qw[:, _QIDX, :].astype(bf))        # [B,128,NTQ,D]

    nc = get_nc()
    in_maps = []
    for k in range(NCORES):
        sl = slice(k * BPC, (k + 1) * BPC)
        in_maps.append({
            "C_bf": np.ascontiguousarray(C_bf[sl]),
            "Qm_bf": np.ascontiguousarray(Qm[sl]),
            "QwP_bf": QwP[sl],
            "sub0c_f": sub0c[sl],
            "wcol_f": np.ascontiguousarray(wcol[sl]),
        })
    trace = bool(int(os.environ.get("BASS_KERNEL_TRACE", "0")))
    res = run_bass_kernel_spmd(nc, in_maps, list(range(NCORES)), trace=trace)
    LAST_RESULT = res

    acb = np.concatenate([np.asarray(res.results[k]["OUT"]) for k in range(NCORES)],
                         axis=0).astype(np.float32)          # [B, Lc, 384]
    out = np.empty((B, Lc, 4 * D), dtype=np.float32)
    out[..., 0:D] = C
    out[..., D:] = acb
    return out



# revision 24
# speedup vs baseline: 1.7823x; 1.7823x over previous
"""Trainium2 Bass kernel for nn_COAttention (trilinear co-attention).

Math (per batch, masks are all-ones by problem spec, bias cancels in softmax):
  S    = C@w4C + (Q@w4Q)^T + (C*w4mlu)@Q^T          [Lc, Lq]
  S1   = softmax(S, axis=q) ; S2 = softmax(S, axis=c)
  A    = S1@Q ; Bt = (S1@S2^T)@C = S1@(S2^T@C)      (reassociated)
  out  = concat([C, A, C*A, C*Bt], -1)

Device formulation (single exp pass; exp(sub1) factors cancel in the S2 path):
  E2[c,q] = exp(sub2[c,q] + sub0[c])     (sub0 as per-partition ACT bias)
  w[q]    = exp(sub1[q])                 (host-computed, tiny)
  T'      = (E2^T @ C) / (E2^T @ 1)      == S2^T@C exactly
  [Ab|Bb|r] = E2 @ [Q*w | T'*w | w] ; A = Ab/r ; Bt = Bb/r

Schedule (v2): host pre-transposes C^T/Qm^T (no device input transposes),
all DRAM layouts strip-packed [128, tiles, ...] for wide DMA descriptors.
PE warm-up transposes during the load lead-in keep the PE p-state ramped.
Phase A (m1 matmul -> ACT exp -> PE transposes+Gt, DVE PSUM->SBUF copies
with col-sum accum) runs with a lag-1 half-group interleave sized for zero
PE stalls (ps_s=3 PSUM bufs). Phase B (m3) epilogue is DVE/Pool only (ACT
does exp exclusively -- no activation-table switches on HW). Batch 1's
phase A is zipped with batch 0's phase B; outputs DMA out per 4-tile group.
PSUM banks: ps_s 3 + ps_f 3 + ps_tt 1 (ping-pong halves) + ps_g 1 = 8.

Host prep (~0.05% of FLOPs): sub0=C@w4C, w=exp(Q@w4Q), CT/QmT/Qw packs,
bf16 casts. Output: device returns [A|C*A|C*Bt] bf16; host prepends exact C.

Sharding: data-parallel over batch, 2 batches per core on 8 cores.
"""

import os
import sys

if "/opt/trn_rl_repo" not in sys.path:
    sys.path.insert(0, "/opt/trn_rl_repo")

import numpy as np
import ml_dtypes

from concourse import bacc, bass, mybir, tile
from concourse.bass_utils import run_bass_kernel_spmd
from concourse.masks import make_identity

F32 = mybir.dt.float32
BF16 = mybir.dt.bfloat16
EXP = mybir.ActivationFunctionType.Exp
COPY = mybir.ActivationFunctionType.Copy
MULT = mybir.AluOpType.mult
ADD = mybir.AluOpType.add
AX = mybir.AxisListType.X

B, Lc, Lq, D = 16, 2048, 512, 128
NCORES = 8
BPC = B // NCORES          # batches per core
NTC = Lc // 128            # 16 c-tiles
NTQ = Lq // 128            # 4 q-tiles
NWARM = 48                 # PE warm-up transposes (cover p-state ramp)

_NC_CACHE = {}
LAST_RESULT = None


def _body(tc, nc, Cd, CTd, QmTd, Qwd, SWd, OUT):
    with (
        tc.tile_pool(name="const", bufs=1) as constp,
        tc.tile_pool(name="io", bufs=2) as iop,
        tc.tile_pool(name="big", bufs=2) as bigp,
        tc.tile_pool(name="small", bufs=2) as smallp,
        tc.tile_pool(name="ep", bufs=4) as epp,
        tc.tile_pool(name="ps_s", bufs=3, space="PSUM") as ps_s,
        tc.tile_pool(name="ps_x", bufs=4, space="PSUM") as ps_x,
        tc.tile_pool(name="ps_g", bufs=1, space="PSUM") as ps_g,
    ):
        ident = constp.tile([128, 128], BF16)
        make_identity(nc, ident[:])

        # ps_x: one shared 4-bank rotation — phase A transposes (tag-shared
        # with) phase B m3 accumulators, so the m3 tail gets 4-deep PSUM
        # pipelining without exceeding the 8-bank budget.
        def next_tslot():
            return ps_x.tile([128, 4, 128], BF16, tag="x", name="t")

        # PE warm-up: dependency-free transposes to ramp the p-state while
        # the input DMAs stream in.  They write transpose scratch; real
        # users are ordered after them by the tile framework.
        def ph_warmup():
            for w in range(NWARM):
                wt = next_tslot()
                nc.tensor.transpose(wt[:, 0, :], ident[:], ident[:])

        st = [dict() for _ in range(BPC)]

        def ph_load(b):
            s = st[b]
            s["ct"] = iop.tile([128, NTC, 128], BF16, tag="ct", name="ct")
            nc.sync.dma_start(s["ct"][:, 0:4, :], CTd[b][:, 0:4, :])
            s["qmt"] = iop.tile([128, NTQ, 128], BF16, tag="qmt", name="qmt")
            nc.sync.dma_start(s["qmt"][:], QmTd[b])
            s["swcol"] = smallp.tile([128, NTC + NTQ], F32, tag="swc",
                                     name="swc")
            nc.sync.dma_start(s["swcol"][:], SWd[b][:, :])
            s["sub0_col"] = s["swcol"][:, 0:NTC]
            s["w_col"] = s["swcol"][:, NTC : NTC + NTQ]
            nc.sync.dma_start(s["ct"][:, 4:16, :], CTd[b][:, 4:16, :])
            s["c_bf"] = iop.tile([128, NTC, 128], BF16, tag="c_bf", name="c_bf")
            nc.sync.dma_start(s["c_bf"][:], Cd[b])
            s["rhs_t"] = bigp.tile([128, NTQ, 257], BF16, tag="rhs", name="rhs")
            nc.sync.dma_start(s["rhs_t"][:, :, 0:128], Qwd[b])
            s["e2n"] = bigp.tile([128, NTC, 512], BF16, tag="e2n", name="e2n")
            s["e2t"] = bigp.tile([128, NTQ, NTC, 128], BF16, tag="e2t", name="e2t")
            s["spart"] = smallp.tile([128, NTQ, 4], F32, tag="spart", name="spart")
            # out columns: [A | Bt scratch | C*A | C*Bt]; host slices out Bt
            s["out_sb"] = bigp.tile([128, NTC, 512], BF16, tag="osb", name="osb")

        def ph_m1_pair(b, i0):
            # S matmul + exp for c-tiles i0, i0+1
            s = st[b]
            for i in (i0, i0 + 1):
                s_ps = ps_s.tile([128, 512], F32, tag="s", name="s")
                nc.tensor.matmul(s_ps[:], lhsT=s["ct"][:, i, :], rhs=s["qmt"][:],
                                 start=True, stop=True)
                nc.scalar.activation(s["e2n"][:, i, :], s_ps[:], EXP,
                                     bias=s["sub0_col"][:, i : i + 1])

        def ph_gt2(b, k, half):
            # 2 of c-group k's Gt accumulation matmuls
            s = st[b]
            if k == 0 and half == 0:
                s["gt_ps"] = ps_g.tile([128, 512], F32, tag="g", name="g")
            for m in (2 * half, 2 * half + 1):
                i = k * 4 + m
                nc.tensor.matmul(s["gt_ps"][:], lhsT=s["c_bf"][:, i, :],
                                 rhs=s["e2n"][:, i, :],
                                 start=(i == 0), stop=(i == NTC - 1))

        def ph_T_half(b, k, half):
            # E2^T via PE transposes (+ col-sum accum) for q-tiles (2*half,
            # 2*half+1) of c-group k
            s = st[b]
            for j in (2 * half, 2 * half + 1):
                t_ps = next_tslot()
                for m in range(4):
                    i = k * 4 + m
                    nc.tensor.transpose(t_ps[:, m, :],
                                        s["e2n"][:, i, j * 128 : (j + 1) * 128],
                                        ident[:])
                dst = s["e2t"][:, j, k * 4 : (k + 1) * 4, :]
                nc.vector.tensor_scalar(
                    out=dst, in0=t_ps[:], scalar1=1.0, scalar2=None,
                    op0=MULT, op1=ADD,
                    accum_out=s["spart"][:, j, k : k + 1])

        def ph_tgt_half(b, k, half):
            ph_gt2(b, k, half)
            ph_T_half(b, k, half)

        def ph_trhs1(b):
            # Gt PSUM -> SBUF copy; emitted right after the last Gt matmul so
            # it overlaps the final transpose flush on PE.
            s = st[b]
            s["gt_bf"] = smallp.tile([128, 512], BF16, tag="gtbf", name="gtbf")
            nc.vector.tensor_copy(s["gt_bf"][:], s["gt_ps"][:])

        def ph_trhs2(b):
            s = st[b]
            gt_bf = s["gt_bf"]
            s_col = smallp.tile([128, NTQ], F32, tag="scol", name="scol")
            nc.vector.reduce_sum(s_col[:], s["spart"][:], axis=AX)
            rs_col = smallp.tile([128, NTQ], F32, tag="rscol", name="rscol")
            nc.vector.reciprocal(rs_col[:], s_col[:])
            ws_col = smallp.tile([128, NTQ], F32, tag="wscol", name="wscol")
            nc.vector.tensor_mul(ws_col[:], s["w_col"][:], rs_col[:])

            for j in range(NTQ):
                gt_tp = next_tslot()
                nc.tensor.transpose(gt_tp[:, 0, :],
                                    gt_bf[:, j * 128 : (j + 1) * 128], ident[:])
                nc.vector.tensor_scalar_mul(s["rhs_t"][:, j, 128:256],
                                            gt_tp[:, 0, :],
                                            ws_col[:, j : j + 1])
                nc.vector.tensor_copy(s["rhs_t"][:, j, 256:257],
                                      s["w_col"][:, j : j + 1])

        def ph_m3_pair(b, i0):
            s = st[b]
            fps = []
            for i in (i0, i0 + 1):
                f_ps = ps_x.tile([128, 257], F32, tag="x", name="f")
                for j in range(NTQ):
                    nc.tensor.matmul(f_ps[:], lhsT=s["e2t"][:, j, i, :],
                                     rhs=s["rhs_t"][:, j, :],
                                     start=(j == 0), stop=(j == NTQ - 1))
                fps.append(f_ps)
            for t, i in enumerate((i0, i0 + 1)):
                rr = epp.tile([128, 1], F32, tag="rr", name="rr")
                nc.vector.reciprocal(rr[:], fps[t][:, 256:257])
                nc.scalar.activation(s["out_sb"][:, i, 0:256],
                                     fps[t][:, 0:256], COPY, scale=rr[:])  # A|Bt
            nc.gpsimd.tensor_mul(s["out_sb"][:, i0 : i0 + 2, 256:384],
                                 s["out_sb"][:, i0 : i0 + 2, 0:128],
                                 s["c_bf"][:, i0 : i0 + 2, :])           # C*A
            nc.vector.tensor_mul(s["out_sb"][:, i0 : i0 + 2, 384:512],
                                 s["out_sb"][:, i0 : i0 + 2, 128:256],
                                 s["c_bf"][:, i0 : i0 + 2, :])           # C*Bt
            if i0 % 4 == 2:
                g = i0 // 4
                nc.sync.dma_start(OUT[b][:, g * 4 : (g + 1) * 4, :],
                                  s["out_sb"][:, g * 4 : (g + 1) * 4, :])

        def ph_a_group(b, k):
            ph_m1_pair(b, 4 * k)
            if k > 0:
                ph_tgt_half(b, k - 1, 0)
            elif b == 0:
                for _ in range(6):   # keep PE dense at the very start
                    wt = next_tslot()
                    nc.tensor.transpose(wt[:, 0, :], ident[:], ident[:])
            ph_m1_pair(b, 4 * k + 2)
            if k > 0:
                ph_tgt_half(b, k - 1, 1)

        # ---- schedule ----
        ph_warmup()
        ph_load(0)
        ph_load(1)

        for k in range(4):
            ph_a_group(0, k)

        # boundary: b0's Gt finish + transpose flush ride between b1's first
        # m1 pairs; the gt_bf copy overlaps the transpose flush on PE.
        ph_m1_pair(1, 0)
        ph_gt2(0, 3, 0)
        ph_gt2(0, 3, 1)
        ph_trhs1(0)
        ph_m1_pair(1, 2)
        ph_T_half(0, 3, 0)
        ph_T_half(0, 3, 1)
        ph_trhs2(0)

        # zip batch 1 phase A with batch 0 phase B.  k0 carries no transpose
        # work (lag-1); two pairs are held back to cover the trhs(1) latency
        # gap before phase B(1) can start.
        b0_pairs = list(range(0, NTC, 2))      # 8 m3 pairs
        zi = 0

        def zip_b0():
            nonlocal zi
            if zi < len(b0_pairs):
                ph_m3_pair(0, b0_pairs[zi])
                zi += 1

        zip_b0()
        for k in range(1, 4):
            ph_m1_pair(1, 4 * k)
            ph_tgt_half(1, k - 1, 0)
            zip_b0()
            ph_m1_pair(1, 4 * k + 2)
            ph_tgt_half(1, k - 1, 1)
            if k < 3:
                zip_b0()
        ph_gt2(1, 3, 0)
        ph_gt2(1, 3, 1)
        ph_trhs1(1)
        zip_b0()
        ph_T_half(1, 3, 0)
        ph_T_half(1, 3, 1)
        ph_trhs2(1)
        while zi < len(b0_pairs):
            zip_b0()

        for i0 in range(0, NTC, 2):
            ph_m3_pair(1, i0)


def _build_nc(n_iters=1):
    nc = bacc.Bacc("TRN2", target_bir_lowering=False, debug=False)
    Cd = nc.declare_dram_parameter("C_bf", [BPC, 128, NTC, D], BF16,
                                   isOutput=False)
    CTd = nc.declare_dram_parameter("CT_bf", [BPC, 128, NTC, 128], BF16,
                                    isOutput=False)
    QmTd = nc.declare_dram_parameter("QmT_bf", [BPC, 128, NTQ, 128], BF16,
                                     isOutput=False)
    Qwd = nc.declare_dram_parameter("QwP_bf", [BPC, 128, NTQ, D], BF16,
                                    isOutput=False)
    SWd = nc.declare_dram_parameter("swcol_f", [BPC, 128, NTC + NTQ], F32,
                                    isOutput=False)
    OUT = nc.declare_dram_parameter("OUT", [BPC, 128, NTC, 4 * D], BF16,
                                    isOutput=True)
    with tile.TileContext(nc) as tc:
        if n_iters == 1:
            _body(tc, nc, Cd, CTd, QmTd, Qwd, SWd, OUT)
        else:
            hints = (mybir.EngineType.PE, mybir.EngineType.DVE,
                     mybir.EngineType.Activation, mybir.EngineType.Pool,
                     mybir.EngineType.SP)
            with tc.For_i(0, n_iters, 1, hint_engines=hints):
                _body(tc, nc, Cd, CTd, QmTd, Qwd, SWd, OUT)
    nc.compile()
    return nc


def get_nc():
    if "nc" not in _NC_CACHE:
        _NC_CACHE["nc"] = _build_nc()
    return _NC_CACHE["nc"]


def prep_in_maps(C, Q, w4C, w4Q, w4mlu):
    """Host prep: rank-1 bias terms, transposed/strip-packed bf16 layouts."""
    bf = ml_dtypes.bfloat16
    C = np.asarray(C, dtype=np.float32)
    Q = np.asarray(Q, dtype=np.float32)
    w4C = np.asarray(w4C, dtype=np.float32).reshape(D)
    w4Q = np.asarray(w4Q, dtype=np.float32).reshape(D)
    w4mlu = np.asarray(w4mlu, dtype=np.float32).reshape(D)

    sub0 = C @ w4C                                   # [B, Lc]
    w = np.exp(Q @ w4Q)                              # [B, Lq]
    Qm = (Q * w4mlu).astype(bf)                      # [B, Lq, D]
    Qw = Q * w[:, :, None]                           # [B, Lq, D]

    Cn = np.ascontiguousarray(
        C.reshape(B, NTC, 128, D).transpose(0, 2, 1, 3).astype(bf))
    CT = np.ascontiguousarray(
        C.transpose(0, 2, 1).reshape(B, 128, NTC, 128).astype(bf))
    QmT = np.ascontiguousarray(
        Qm.transpose(0, 2, 1).reshape(B, 128, NTQ, 128))
    QwP = np.ascontiguousarray(
        Qw.reshape(B, NTQ, 128, D).transpose(0, 2, 1, 3).astype(bf))
    sub0c = sub0.reshape(B, NTC, 128).transpose(0, 2, 1)
    wcol = w.reshape(B, NTQ, 128).transpose(0, 2, 1)
    swcol = np.ascontiguousarray(
        np.concatenate([sub0c, wcol], axis=2).astype(np.float32))

    in_maps = []
    for k in range(NCORES):
        sl = slice(k * BPC, (k + 1) * BPC)
        in_maps.append({
            "C_bf": Cn[sl],
            "CT_bf": CT[sl],
            "QmT_bf": QmT[sl],
            "QwP_bf": QwP[sl],
            "swcol_f": swcol[sl],
        })
    return in_maps


def kernel(C, Q, Cmask=None, Qmask=None, w4C=None, w4Q=None, w4mlu=None,
           bias=None, **_unused):
    """Full inputs in, full output out. Masks are all-ones (problem spec);
    bias is a scalar added to S pre-softmax, which cancels in both softmaxes."""
    global LAST_RESULT
    C = np.asarray(C, dtype=np.float32)
    in_maps = prep_in_maps(C, Q, w4C, w4Q, w4mlu)

    nc = get_nc()
    trace = bool(int(os.environ.get("BASS_KERNEL_TRACE", "0")))
    res = run_bass_kernel_spmd(nc, in_maps, list(range(NCORES)), trace=trace)
    LAST_RESULT = res

    acb = np.concatenate(
        [np.asarray(res.results[k]["OUT"]) for k in range(NCORES)],
        axis=0).astype(np.float32)                   # [B, 128, NTC, 512]
    acb = acb.transpose(0, 2, 1, 3).reshape(B, Lc, 4 * D)
    out = np.empty((B, Lc, 4 * D), dtype=np.float32)
    out[..., 0:D] = C
    out[..., D : 2 * D] = acb[..., 0:D]              # A
    out[..., 2 * D :] = acb[..., 2 * D :]            # C*A, C*Bt
    return out


# revision 52
# speedup vs baseline: 1.9298x; 1.0828x over previous
"""Trainium2 Bass kernel for nn_COAttention (trilinear co-attention).

Math (per batch, masks are all-ones by problem spec, bias cancels in softmax):
  S    = C@w4C + (Q@w4Q)^T + (C*w4mlu)@Q^T          [Lc, Lq]
  S1   = softmax(S, axis=q) ; S2 = softmax(S, axis=c)
  A    = S1@Q ; Bt = (S1@S2^T)@C = S1@(S2^T@C)      (reassociated)
  out  = concat([C, A, C*A, C*Bt], -1)

Device formulation (single exp pass; exp(sub1) factors cancel in the S2 path):
  E2[c,q] = exp(sub2[c,q] + sub0[c])     (sub0 as per-partition ACT bias)
  w[q]    = exp(sub1[q])                 (host-computed, tiny)
  T'      = (E2^T @ C) / (E2^T @ 1)      == S2^T@C exactly
  [Ab|Bb|r] = E2 @ [Q*w | T'*w | w] ; A = Ab/r ; Bt = Bb/r

Schedule (v3): host pre-transposes C^T/Qm^T (no device input transposes),
all DRAM layouts strip-packed [128, tiles, ...] so every DMA descriptor is
>=1KB contiguous.  PE warm-up transposes during the load lead-in keep the
PE p-state ramped (TRN2 runs matmuls 2-4x slower for ~3us after any PE
idle).  Phase A per batch: m1 matmul -> ACT exp (sub0 as bias) -> lag-1
half-group interleave of PE transposes (E2^T) + Gt matmuls, with DVE
PSUM->SBUF copies carrying the S2-denominator column-sum accumulation;
ps_s=3 PSUM bufs make it stall-free.  Phase B per c-tile pair: 4+4 m3
matmuls -> DVE recip -> one fused ACT scale ([A|Bt], 256 wide) -> paired
Pool/DVE muls -> per-4-tile-group output DMA.  Batch 1's phase A is zipped
with batch 0's phase B so PE/ACT/DVE/Pool stay loaded throughout.
PSUM banks: ps_s 3 + ps_x 4 (shared: A-phase transpose scratch / B-phase
m3 accumulators) + ps_g 1 = 8.

Host prep (~0.05% of FLOPs): sub0=C@w4C, w=exp(Q@w4Q), CT/QmT/Qw packs,
bf16 casts.  Output: device returns [A|Bt|C*A|C*Bt] bf16 (Bt is scratch
the host drops -- storing it beats a WAR chain); host prepends exact C.

Sharding: data-parallel over batch, 2 batches per core on 8 cores.
"""

import os
import sys

if "/opt/trn_rl_repo" not in sys.path:
    sys.path.insert(0, "/opt/trn_rl_repo")

import numpy as np
import ml_dtypes

from concourse import bacc, bass, mybir, tile
from concourse.bass_utils import run_bass_kernel_spmd
from concourse.masks import make_identity

F32 = mybir.dt.float32
BF16 = mybir.dt.bfloat16
EXP = mybir.ActivationFunctionType.Exp
COPY = mybir.ActivationFunctionType.Copy
MULT = mybir.AluOpType.mult
ADD = mybir.AluOpType.add
AX = mybir.AxisListType.X

B, Lc, Lq, D = 16, 2048, 512, 128
NCORES = 8
BPC = B // NCORES          # batches per core
NTC = Lc // 128            # 16 c-tiles
NTQ = Lq // 128            # 4 q-tiles
NWARM = 48                 # PE warm-up transposes (cover p-state ramp)

_NC_CACHE = {}
LAST_RESULT = None


def _body(tc, nc, Cd, CTd, QmTd, Qwd, SWd, OUT):
    with (
        tc.tile_pool(name="const", bufs=1) as constp,
        tc.tile_pool(name="io", bufs=2) as iop,
        tc.tile_pool(name="big", bufs=2) as bigp,
        tc.tile_pool(name="small", bufs=2) as smallp,
        tc.tile_pool(name="ep", bufs=4) as epp,
        tc.tile_pool(name="ps_s", bufs=3, space="PSUM") as ps_s,
        tc.tile_pool(name="ps_x", bufs=4, space="PSUM") as ps_x,
        tc.tile_pool(name="ps_g", bufs=1, space="PSUM") as ps_g,
    ):
        ident = constp.tile([128, 128], BF16)
        make_identity(nc, ident[:])

        # ps_x: one shared 4-bank rotation — phase A transposes (tag-shared
        # with) phase B m3 accumulators, so the m3 tail gets 4-deep PSUM
        # pipelining without exceeding the 8-bank budget.
        def next_tslot():
            return ps_x.tile([128, 4, 128], BF16, tag="x", name="t")

        # PE warm-up: dependency-free transposes to ramp the p-state while
        # the input DMAs stream in.  They write transpose scratch; real
        # users are ordered after them by the tile framework.
        def ph_warmup():
            for w in range(NWARM):
                wt = next_tslot()
                nc.tensor.transpose(wt[:, 0, :], ident[:], ident[:])

        st = [dict() for _ in range(BPC)]

        def ph_load(b):
            s = st[b]
            s["ct"] = iop.tile([128, NTC, 128], BF16, tag="ct", name="ct")
            nc.sync.dma_start(s["ct"][:, 0:4, :], CTd[b][:, 0:4, :])
            s["qmt"] = iop.tile([128, NTQ, 128], BF16, tag="qmt", name="qmt")
            nc.sync.dma_start(s["qmt"][:], QmTd[b])
            s["swcol"] = smallp.tile([128, NTC + NTQ], F32, tag="swc",
                                     name="swc")
            nc.sync.dma_start(s["swcol"][:], SWd[b][:, :])
            s["sub0_col"] = s["swcol"][:, 0:NTC]
            s["w_col"] = s["swcol"][:, NTC : NTC + NTQ]
            nc.sync.dma_start(s["ct"][:, 4:16, :], CTd[b][:, 4:16, :])
            s["c_bf"] = iop.tile([128, NTC, 128], BF16, tag="c_bf", name="c_bf")
            nc.sync.dma_start(s["c_bf"][:], Cd[b])
            s["rhs_t"] = bigp.tile([128, NTQ, 257], BF16, tag="rhs", name="rhs")
            nc.sync.dma_start(s["rhs_t"][:, :, 0:128], Qwd[b])
            s["e2n"] = bigp.tile([128, NTC, 512], BF16, tag="e2n", name="e2n")
            s["e2t"] = bigp.tile([128, NTQ, NTC, 128], BF16, tag="e2t", name="e2t")
            s["spart"] = smallp.tile([128, NTQ, 4], F32, tag="spart", name="spart")
            # out columns: [A | Bt scratch | C*A | C*Bt]; host slices out Bt.
            # (An extra 128 stored cols beats reusing [128:256] as scratch:
            # the WAR chain mulBt->mulA costs ~2us in HW sem latency.)
            s["out_sb"] = bigp.tile([128, NTC, 512], BF16, tag="osb", name="osb")

        def ph_m1_pair(b, i0):
            # S matmul + exp for c-tiles i0, i0+1
            s = st[b]
            for i in (i0, i0 + 1):
                s_ps = ps_s.tile([128, 512], F32, tag="s", name="s")
                nc.tensor.matmul(s_ps[:], lhsT=s["ct"][:, i, :], rhs=s["qmt"][:],
                                 start=True, stop=True)
                nc.scalar.activation(s["e2n"][:, i, :], s_ps[:], EXP,
                                     bias=s["sub0_col"][:, i : i + 1])

        def ph_gt2(b, k, half):
            # 2 of c-group k's Gt accumulation matmuls.  (One open
            # accumulation group per PSUM bank: computing G^T directly as 4
            # concurrently-open per-q-tile groups in one bank corrupts the
            # accumulation on HW.)
            s = st[b]
            if k == 0 and half == 0:
                s["gt_ps"] = ps_g.tile([128, 512], F32, tag="g", name="g")
            for m in (2 * half, 2 * half + 1):
                i = k * 4 + m
                nc.tensor.matmul(s["gt_ps"][:], lhsT=s["c_bf"][:, i, :],
                                 rhs=s["e2n"][:, i, :],
                                 start=(i == 0), stop=(i == NTC - 1))

        def ph_T_half(b, k, half):
            # E2^T via PE transposes (+ col-sum accum) for q-tiles (2*half,
            # 2*half+1) of c-group k
            s = st[b]
            for j in (2 * half, 2 * half + 1):
                t_ps = next_tslot()
                for m in range(4):
                    i = k * 4 + m
                    nc.tensor.transpose(t_ps[:, m, :],
                                        s["e2n"][:, i, j * 128 : (j + 1) * 128],
                                        ident[:])
                dst = s["e2t"][:, j, k * 4 : (k + 1) * 4, :]
                nc.vector.tensor_scalar(
                    out=dst, in0=t_ps[:], scalar1=1.0, scalar2=None,
                    op0=MULT, op1=ADD,
                    accum_out=s["spart"][:, j, k : k + 1])

        def ph_tgt_half(b, k, half):
            ph_gt2(b, k, half)
            ph_T_half(b, k, half)

        def ph_trhs1(b):
            # Gt PSUM -> SBUF copy; emitted right after the last Gt matmul so
            # it overlaps the final transpose flush on PE.
            s = st[b]
            s["gt_bf"] = smallp.tile([128, 512], BF16, tag="gtbf", name="gtbf")
            nc.vector.tensor_copy(s["gt_bf"][:], s["gt_ps"][:])

        def ph_trhs2(b):
            s = st[b]
            s_col = smallp.tile([128, NTQ], F32, tag="scol", name="scol")
            nc.vector.reduce_sum(s_col[:], s["spart"][:], axis=AX)
            rs_col = smallp.tile([128, NTQ], F32, tag="rscol", name="rscol")
            nc.vector.reciprocal(rs_col[:], s_col[:])
            ws_col = smallp.tile([128, NTQ], F32, tag="wscol", name="wscol")
            nc.vector.tensor_mul(ws_col[:], s["w_col"][:], rs_col[:])

            for j in range(NTQ):
                gt_tp = next_tslot()
                nc.tensor.transpose(gt_tp[:, 0, :],
                                    s["gt_bf"][:, j * 128 : (j + 1) * 128],
                                    ident[:])
                nc.vector.tensor_scalar_mul(s["rhs_t"][:, j, 128:256],
                                            gt_tp[:, 0, :],
                                            ws_col[:, j : j + 1])
                nc.vector.tensor_copy(s["rhs_t"][:, j, 256:257],
                                      s["w_col"][:, j : j + 1])

        def ph_m3_pair(b, i0):
            s = st[b]
            fps = []
            for i in (i0, i0 + 1):
                f_ps = ps_x.tile([128, 257], F32, tag="x", name="f")
                for j in range(NTQ):
                    nc.tensor.matmul(f_ps[:], lhsT=s["e2t"][:, j, i, :],
                                     rhs=s["rhs_t"][:, j, :],
                                     start=(j == 0), stop=(j == NTQ - 1))
                fps.append(f_ps)
            for t, i in enumerate((i0, i0 + 1)):
                rr = epp.tile([128, 1], F32, tag="rr", name="rr")
                nc.vector.reciprocal(rr[:], fps[t][:, 256:257])
                nc.scalar.activation(s["out_sb"][:, i, 0:256],
                                     fps[t][:, 0:256], COPY, scale=rr[:])  # A|Bt
            nc.vector.tensor_mul(s["out_sb"][:, i0 : i0 + 2, 384:512],
                                 s["out_sb"][:, i0 : i0 + 2, 128:256],
                                 s["c_bf"][:, i0 : i0 + 2, :])           # C*Bt
            nc.gpsimd.tensor_mul(s["out_sb"][:, i0 : i0 + 2, 256:384],
                                 s["out_sb"][:, i0 : i0 + 2, 0:128],
                                 s["c_bf"][:, i0 : i0 + 2, :])           # C*A
            if b == 1 and i0 >= 12:
                # split the final stores so the drain tail is one 2-tile DMA
                nc.sync.dma_start(OUT[b][:, i0 : i0 + 2, :],
                                  s["out_sb"][:, i0 : i0 + 2, :])
            elif i0 % 4 == 2:
                g = i0 // 4
                nc.sync.dma_start(OUT[b][:, g * 4 : (g + 1) * 4, :],
                                  s["out_sb"][:, g * 4 : (g + 1) * 4, :])

        def ph_a_group(b, k):
            ph_m1_pair(b, 4 * k)
            if k > 0:
                ph_tgt_half(b, k - 1, 0)
            elif b == 0:
                for _ in range(18):  # k0 has no lagged work: pad PE while the
                    wt = next_tslot()   # first exps drain (ACT-paced start)
                    nc.tensor.transpose(wt[:, 0, :], ident[:], ident[:])
            ph_m1_pair(b, 4 * k + 2)
            if k > 0:
                ph_tgt_half(b, k - 1, 1)

        # ---- schedule ----
        ph_warmup()
        ph_load(0)
        ph_load(1)

        for k in range(4):
            ph_a_group(0, k)

        # boundary: b0's Gt finish + transpose flush ride between b1's first
        # m1 pairs; the gt_bf copy overlaps the transpose flush on PE.
        ph_m1_pair(1, 0)
        ph_gt2(0, 3, 0)
        ph_gt2(0, 3, 1)
        pass  # trhs1 removed
        ph_m1_pair(1, 2)
        ph_T_half(0, 3, 0)
        ph_T_half(0, 3, 1)
        ph_trhs2(0)

        # zip batch 1 phase A with batch 0 phase B.  k0 carries no transpose
        # work (lag-1); two pairs are held back to cover the trhs(1) latency
        # gap before phase B(1) can start.
        b0_pairs = list(range(0, NTC, 2))      # 8 m3 pairs
        zi = 0

        def zip_b0():
            nonlocal zi
            if zi < len(b0_pairs):
                ph_m3_pair(0, b0_pairs[zi])
                zi += 1

        zip_b0()
        for k in range(1, 4):
            ph_m1_pair(1, 4 * k)
            ph_tgt_half(1, k - 1, 0)
            zip_b0()
            ph_m1_pair(1, 4 * k + 2)
            ph_tgt_half(1, k - 1, 1)
            if k < 3:
                zip_b0()
        ph_gt2(1, 3, 0)
        ph_gt2(1, 3, 1)
        pass  # trhs1 removed
        zip_b0()
        ph_T_half(1, 3, 0)
        ph_T_half(1, 3, 1)
        ph_trhs2(1)
        while zi < len(b0_pairs):
            zip_b0()

        for i0 in range(0, NTC, 2):
            ph_m3_pair(1, i0)


def _build_nc(n_iters=1):
    nc = bacc.Bacc("TRN2", target_bir_lowering=False, debug=False)
    Cd = nc.declare_dram_parameter("C_bf", [BPC, 128, NTC, D], BF16,
                                   isOutput=False)
    CTd = nc.declare_dram_parameter("CT_bf", [BPC, 128, NTC, 128], BF16,
                                    isOutput=False)
    QmTd = nc.declare_dram_parameter("QmT_bf", [BPC, 128, NTQ, 128], BF16,
                                     isOutput=False)
    Qwd = nc.declare_dram_parameter("QwP_bf", [BPC, 128, NTQ, D], BF16,
                                    isOutput=False)
    SWd = nc.declare_dram_parameter("swcol_f", [BPC, 128, NTC + NTQ], F32,
                                    isOutput=False)
    OUT = nc.declare_dram_parameter("OUT", [BPC, 128, NTC, 4 * D], BF16,
                                    isOutput=True)
    with tile.TileContext(nc) as tc:
        if n_iters == 1:
            _body(tc, nc, Cd, CTd, QmTd, Qwd, SWd, OUT)
        else:
            hints = (mybir.EngineType.PE, mybir.EngineType.DVE,
                     mybir.EngineType.Activation, mybir.EngineType.Pool,
                     mybir.EngineType.SP)
            with tc.For_i(0, n_iters, 1, hint_engines=hints):
                _body(tc, nc, Cd, CTd, QmTd, Qwd, SWd, OUT)
    nc.compile()
    return nc


def get_nc():
    if "nc" not in _NC_CACHE:
        _NC_CACHE["nc"] = _build_nc()
    return _NC_CACHE["nc"]


def prep_in_maps(C, Q, w4C, w4Q, w4mlu):
    """Host prep: rank-1 bias terms, transposed/strip-packed bf16 layouts."""
    bf = ml_dtypes.bfloat16
    C = np.asarray(C, dtype=np.float32)
    Q = np.asarray(Q, dtype=np.float32)
    w4C = np.asarray(w4C, dtype=np.float32).reshape(D)
    w4Q = np.asarray(w4Q, dtype=np.float32).reshape(D)
    w4mlu = np.asarray(w4mlu, dtype=np.float32).reshape(D)

    sub0 = C @ w4C                                   # [B, Lc]
    w = np.exp(Q @ w4Q)                              # [B, Lq]
    Qm = (Q * w4mlu).astype(bf)                      # [B, Lq, D]
    Qw = Q * w[:, :, None]                           # [B, Lq, D]

    Cn = np.ascontiguousarray(
        C.reshape(B, NTC, 128, D).transpose(0, 2, 1, 3).astype(bf))
    CT = np.ascontiguousarray(
        C.transpose(0, 2, 1).reshape(B, 128, NTC, 128).astype(bf))
    QmT = np.ascontiguousarray(
        Qm.transpose(0, 2, 1).reshape(B, 128, NTQ, 128))
    QwP = np.ascontiguousarray(
        Qw.reshape(B, NTQ, 128, D).transpose(0, 2, 1, 3).astype(bf))
    sub0c = sub0.reshape(B, NTC, 128).transpose(0, 2, 1)
    wcol = w.reshape(B, NTQ, 128).transpose(0, 2, 1)
    swcol = np.ascontiguousarray(
        np.concatenate([sub0c, wcol], axis=2).astype(np.float32))

    in_maps = []
    for k in range(NCORES):
        sl = slice(k * BPC, (k + 1) * BPC)
        in_maps.append({
            "C_bf": Cn[sl],
            "CT_bf": CT[sl],
            "QmT_bf": QmT[sl],
            "QwP_bf": QwP[sl],
            "swcol_f": swcol[sl],
        })
    return in_maps


def kernel(C, Q, Cmask=None, Qmask=None, w4C=None, w4Q=None, w4mlu=None,
           bias=None, **_unused):
    """Full inputs in, full output out. Masks are all-ones (problem spec);
    bias is a scalar added to S pre-softmax, which cancels in both softmaxes."""
    global LAST_RESULT
    C = np.asarray(C, dtype=np.float32)
    in_maps = prep_in_maps(C, Q, w4C, w4Q, w4mlu)

    nc = get_nc()
    trace = bool(int(os.environ.get("BASS_KERNEL_TRACE", "0")))
    res = run_bass_kernel_spmd(nc, in_maps, list(range(NCORES)), trace=trace)
    LAST_RESULT = res

    acb = np.concatenate(
        [np.asarray(res.results[k]["OUT"]) for k in range(NCORES)],
        axis=0).astype(np.float32)                   # [B, 128, NTC, 512]
    acb = acb.transpose(0, 2, 1, 3).reshape(B, Lc, 4 * D)
    out = np.empty((B, Lc, 4 * D), dtype=np.float32)
    out[..., 0:D] = C
    out[..., D : 2 * D] = acb[..., 0:D]              # A
    out[..., 2 * D :] = acb[..., 2 * D :]            # C*A, C*Bt
    return out
